# revision 1
# baseline (speedup 1.0000x reference)
"""Trainium2 Bass kernel: causal spatial attention block (nn_AttentionBlock).

Strategy: data-parallel over batch across 8 NeuronCores (4 batches per core,
no collectives). Per batch: QKV 1x1-conv projections as f32r matmuls,
causal attention computed in [t, s] (keys-on-partitions) orientation so the
probability tiles feed the A@V matmul with no transposes; V is produced
pre-transposed directly by the projection matmul orientation. Softmax
denominators come from a ones-matrix matmul (sum over t broadcast to all
partitions); 1/x is exp(-ln(x)) on the scalar engine (ACT Reciprocal is
banned for accuracy). f32r (TF32-like) matmuls run at 1 cycle/row for
free-dim >= 256; 128-wide diagonal tiles are widened to 256 and masked with
a [zeros | triu] block to stay on the fast path.
"""
import numpy as np
from contextlib import ExitStack

import concourse.bass as bass
import concourse.mybir as mybir
import concourse.tile as tile
from concourse import bacc
from concourse.bass_utils import run_bass_kernel_spmd

F32 = mybir.dt.float32
F32R = mybir.dt.float32r
AF = mybir.ActivationFunctionType
ALU = mybir.AluOpType

B, C, L, EMB = 32, 224, 32, 16
S = L * L            # 1024
CIN = 256
NCORES = 8
NB = B // NCORES     # 4 batches per core


def _pos_embeddings() -> np.ndarray:
    """[2E, S] positional-embedding channels, replicating the reference
    (raw row-major reshape of the [L, E] table, NOT a transpose)."""
    pos = np.arange(L)[:, None].astype(np.float64)
    j = np.arange(EMB)[None, :]
    enc = pos / np.power(10000.0, 2.0 * (j // 2) / EMB)
    enc[0, :] = 0.0
    enc[1:, 0::2] = np.sin(enc[1:, 0::2])
    enc[1:, 1::2] = np.cos(enc[1:, 1::2])
    t = enc.astype(np.float32)                            # [L, E]
    x = np.tile(t.reshape(1, EMB, L, 1), (1, 1, 1, L))
    y = np.tile(t.reshape(1, EMB, 1, L), (1, 1, L, 1))
    pe = np.concatenate((x, y), axis=1)[0]                # [2E, L, L]
    return np.ascontiguousarray(pe.reshape(2 * EMB, S))


def _pin_act_tables():
    """Make Bacc's table picker see only the natural_log_exp_and_others set
    (it holds exp+ln+relu+copy+identity — everything this kernel uses) so a
    single ACT table load serves the whole kernel instead of thrashing
    between exp_and_others and natural_log (~1.3 us per reload)."""
    from concourse import bacc as _bacc
    real = _bacc.get_activation_tables
    def patched(arch):
        tables = real(arch)
        keep = "natural_log_exp_and_others"
        assert keep in tables
        return {name: (funcs if name == keep else set())
                for name, funcs in tables.items()}
    _bacc.get_activation_tables = patched
    return real


def build(reps: int = 1):
    """Build + finalize the per-core Bass program (same program on all 8).

    reps > 1 repeats the whole per-core computation back-to-back inside one
    NEFF — used by the timing harness to amortize launch overhead."""
    real_tables = _pin_act_tables()
    nc = bacc.Bacc("TRN2", target_bir_lowering=False, debug=False,
                   num_devices=NCORES)
    x_d = nc.declare_dram_parameter("x", [NB, C, S], F32, isOutput=False)
    pe_d = nc.declare_dram_parameter("pe", [2 * EMB, S], F32, isOutput=False)
    wq_d = nc.declare_dram_parameter("wqt", [CIN, 256], F32, isOutput=False)
    wk_d = nc.declare_dram_parameter("wkt", [CIN, 256], F32, isOutput=False)
    wv_d = nc.declare_dram_parameter("wvt", [CIN, 256], F32, isOutput=False)
    # bqk: [128, 4] = [bq half0 | bq half1 | bk half0 | bk half1] columns
    bqk_d = nc.declare_dram_parameter("bqk", [128, 4], F32, isOutput=False)
    bv_d = nc.declare_dram_parameter("bv", [256], F32, isOutput=False)
    # mask[:, 0:128] = zeros (unused now), mask[:, 128:256] = triu (t <= s)
    mk_d = nc.declare_dram_parameter("mask", [128, 256], F32, isOutput=False)
    out_d = nc.declare_dram_parameter("out", [NB, 256, S], F32, isOutput=True)

    with ExitStack() as ctx:
        tc = ctx.enter_context(tile.TileContext(nc))
        const = ctx.enter_context(tc.tile_pool(name="const", bufs=1))
        xp = ctx.enter_context(tc.tile_pool(name="x0", bufs=2))
        qkp = ctx.enter_context(tc.tile_pool(name="qk", bufs=2))
        vtp = ctx.enter_context(tc.tile_pool(name="vt", bufs=2))
        pp = ctx.enter_context(tc.tile_pool(name="p", bufs=8))
        ep = ctx.enter_context(tc.tile_pool(name="epi", bufs=4))
        # work: projection + score psum share 5 banks; acc: o0/o1/den 3 banks
        ps_w = ctx.enter_context(tc.tile_pool(name="work", bufs=5, space="PSUM"))
        ps_acc = ctx.enter_context(tc.tile_pool(name="acc", bufs=3, space="PSUM"))

        # -------- batch-0 x load first: it heads the serial DGE queue.
        # Each channel-half is loaded (and f32r-rounded) in two 512-col
        # s-halves so the first projection/attention consumers wait for
        # only a quarter of the batch's data.
        def load_x0(b):
            if "wv" not in consts:
                # wv first: the vT projection is the first compute
                _load_wv()
            x0f = xp.tile([128, 2, S], F32, tag="x0f")
            x0r = xp.tile([128, 2, S], F32R, tag="x0r")
            for h0, h1 in ((0, 512), (512, S)):
                nc.sync.dma_start(x0f[:, 0, h0:h1], x_d[b, 0:128, h0:h1])
                nc.sync.dma_start(x0f[0:96, 1, h0:h1], x_d[b, 128:224, h0:h1])
                nc.sync.dma_start(x0f[96:128, 1, h0:h1], pe_d[:, h0:h1])
                nc.vector.tensor_copy(x0r[:, 0, h0:h1], x0f[:, 0, h0:h1])
                nc.vector.tensor_copy(x0r[:, 1, h0:h1], x0f[:, 1, h0:h1])
            if "wq" not in consts:
                _load_weights()
            return x0r

        consts = {}

        def round_from_dram(dram_ap, shape, name, eng="scalar"):
            f = const.tile(shape, F32, tag=name + "_f")
            nc.sync.dma_start(f[:], dram_ap)
            r = const.tile(shape, F32R, tag=name + "_r")
            if eng == "scalar":
                nc.scalar.copy(r[:], f[:])
            else:
                nc.vector.tensor_copy(r[:], f[:])
            return r

        def _load_wv():
            consts["wv"] = round_from_dram(
                wv_d[:].rearrange("(ci p) m -> p ci m", p=128),
                [128, 2, 256], "wv", eng="vector")
            consts["bv"] = round_from_dram(bv_d[:].unsqueeze(0), [1, 256],
                                           "bv", eng="vector")

        def _make_ones_and_warm():
            # ones needs no DMA: build it immediately, then run throwaway
            # matmuls on it while the startup DMAs stream, so the PE's HAM
            # clock gate is already at 2.4 GHz when real work arrives
            ones_f = const.tile([128, 128], F32, tag="ones_f")
            nc.vector.memset(ones_f[:], 1.0)
            ones_r = const.tile([128, 128], F32R, tag="ones_r")
            nc.vector.tensor_copy(ones_r[:], ones_f[:])
            consts["ones"] = ones_r
            warm = ps_acc.tile([128, 128], F32, tag="acc")
            for _ in range(8):
                nc.tensor.matmul(warm[:], ones_r[:], ones_r[:],
                                 start=True, stop=True)

        def _load_weights():
            consts["wq"] = round_from_dram(
                wq_d[:].rearrange("(ci p) m -> p ci m", p=128),
                [128, 2, 256], "wq")
            consts["wk"] = round_from_dram(
                wk_d[:].rearrange("(ci p) m -> p ci m", p=128),
                [128, 2, 256], "wk")
            consts["mask"] = round_from_dram(mk_d[:], [128, 256], "mask",
                                             eng="vector")
            bqk = const.tile([128, 4], F32, tag="bqk")
            nc.sync.dma_start(bqk[:], bqk_d[:])
            consts["bqk"] = bqk

        _make_ones_and_warm()
        x0r_next = load_x0(0)

        # ---------------- per-batch pipeline ----------------
        seq = [b for _ in range(reps) for b in range(NB)]
        for bi, b in enumerate(seq):
            is_last = (bi == len(seq) - 1)
            x0r = x0r_next
            if not is_last:
                x0r_next = load_x0(seq[bi + 1])

            wq_r, wk_r, wv_r = consts["wq"], consts["wk"], consts["wv"]
            mask_r, bv_r, bqk = consts["mask"], consts["bv"], consts["bqk"]
            ones_r = consts["ones"]

            # V transposed: vT[t, v] = relu(x0.T @ wvT + bv); relus
            # alternate between DVE and ACT so neither engine becomes the
            # drain bottleneck for the work-psum slots.
            vt_r = vtp.tile([128, 8, 256], F32R, tag="vt")
            vt_done = [0]

            def ensure_vt(n):
                for ti in range(vt_done[0], n):
                    ps = ps_w.tile([128, 256], F32, tag="work")
                    for ci in range(2):
                        nc.tensor.matmul(
                            ps[:],
                            x0r[:, ci, ti * 128:(ti + 1) * 128],
                            wv_r[:, ci, :],
                            start=(ci == 0), stop=False)
                    # + bv along the free dim via rank-1 ones x bv update
                    nc.tensor.matmul(ps[:], ones_r[0:1, :], bv_r[0:1, :],
                                     start=False, stop=True)
                    if ti % 2 == 0:
                        nc.vector.tensor_scalar(vt_r[:, ti, :], ps[:], 0.0,
                                                None, op0=ALU.max)
                    else:
                        nc.scalar.activation(vt_r[:, ti, :], ps[:], AF.Relu)
                vt_done[0] = max(vt_done[0], n)

            ensure_vt(8)

            # Q, K projections: q[c_out, s] = relu(wT.T @ x0 + b)
            q_r = qkp.tile([128, 2, S], F32R, tag="q")
            k_r = qkp.tile([128, 2, S], F32R, tag="k")
            for sj in range(2):
                for pi, (wr, dst) in enumerate(((wq_r, q_r), (wk_r, k_r))):
                    for m in range(2):
                        ps = ps_w.tile([128, 512], F32, tag="work")
                        for ci in range(2):
                            nc.tensor.matmul(
                                ps[:],
                                wr[:, ci, m * 128:(m + 1) * 128],
                                x0r[:, ci, sj * 512:(sj + 1) * 512],
                                start=(ci == 0), stop=(ci == 1))
                        # relu(x + b): bias is per-partition; alternate the
                        # consuming engine so the work-psum slots drain via
                        # both DVE and ACT during the projection burst
                        bias_ap = bqk[:, 2 * pi + m:2 * pi + m + 1]
                        dst_ap = dst[:, m, sj * 512:(sj + 1) * 512]
                        if m == 1:
                            nc.scalar.activation(dst_ap, ps[:], AF.Relu,
                                                 bias=bias_ap)
                        else:
                            nc.vector.tensor_scalar(
                                dst_ap, ps[:], bias_ap, 0.0,
                                op0=ALU.add, op1=ALU.max)

            # causal attention over s-chunks; the final batch splits its last
            # 512 columns into two 256-wide chunks so the kernel tail
            # (epilogue of the very last chunk) is half as long
            chunks = [(0, 512), (512, 768), (768, 1024)] if is_last \
                     else [(0, 512), (512, 1024)]
            for (ck0, ck1) in chunks:
                cw = ck1 - ck0
                nt = ck1 // 128
                ensure_vt(nt)
                o0 = ps_acc.tile([128, cw], F32, tag="acc")
                o1 = ps_acc.tile([128, cw], F32, tag="acc")
                dn = ps_acc.tile([128, cw], F32, tag="acc")
                for ti in range(nt):
                    diag = (ti * 128 >= ck0)
                    w = ck1 - max(ck0, ti * 128)
                    we = max(w, 256)       # f32r needs N>=256 for 1 cyc/row
                    cs = ck1 - we
                    loc = cw - we
                    sp = ps_w.tile([128, 512], F32, tag="work")
                    for ci in range(2):
                        nc.tensor.matmul(
                            sp[:, :we],
                            k_r[:, ci, ti * 128:(ti + 1) * 128],
                            q_r[:, ci, cs:cs + we],
                            start=(ci == 0), stop=(ci == 1))
                    p = pp.tile([128, 512], F32R, tag="p")
                    nc.scalar.activation(p[:, :we], sp[:, :we], AF.Exp,
                                         scale=0.0625)
                    if diag and we > w:
                        # widened tile: [zeros | triu] masks both the dead
                        # left half and the diagonal block in one op
                        nc.vector.tensor_tensor(p[:, 0:256], p[:, 0:256],
                                                mask_r[:], op=ALU.mult)
                    elif diag:
                        nc.vector.tensor_tensor(p[:, 0:128], p[:, 0:128],
                                                mask_r[:, 128:256],
                                                op=ALU.mult)
                    first, last = (ti == 0), (ti == nt - 1)
                    # denominator first: its consumer (Ln) gates the epilogue
                    nc.tensor.matmul(dn[:, loc:loc + we], ones_r[:, :],
                                     p[:, :we], start=first, stop=last)
                    nc.tensor.matmul(o0[:, loc:loc + we], vt_r[:, ti, 0:128],
                                     p[:, :we], start=first, stop=last)
                    nc.tensor.matmul(o1[:, loc:loc + we], vt_r[:, ti, 128:256],
                                     p[:, :we], start=first, stop=last)
                # 1/den = exp(-ln(den)); ACT Reciprocal is banned (accuracy)
                # last batch: no more input prefetch — use the fast sync DGE
                dma_eng = nc.sync if is_last else nc.gpsimd
                lnt = ep.tile([128, 512], F32, tag="lnt")
                rec = ep.tile([128, 512], F32, tag="rec")
                nc.scalar.activation(lnt[:, :cw], dn[:], AF.Ln)
                nc.scalar.activation(rec[:, :cw], lnt[:, :cw], AF.Exp,
                                     scale=-1.0)
                for m, om in enumerate((o0, o1)):
                    osb = ep.tile([128, 512], F32, tag="osb")
                    nc.vector.tensor_tensor(osb[:, :cw], om[:], rec[:, :cw],
                                            op=ALU.mult)
                    # outputs ride the Pool engine's DGE mid-kernel to
                    # keep the sync queue free for input prefetch
                    dma_eng.dma_start(
                        out_d[b, m * 128:(m + 1) * 128, ck0:ck1],
                        osb[:, :cw])

    try:
        nc.finalize()
    finally:
        from concourse import bacc as _bacc
        _bacc.get_activation_tables = real_tables
    return nc


def make_in_maps(x, wq, bq, wk, bk, wv, bv):
    x_r = np.ascontiguousarray(x.reshape(B, C, S).astype(np.float32))
    pe = _pos_embeddings()
    wqt = np.ascontiguousarray(wq.T.astype(np.float32))
    wkt = np.ascontiguousarray(wk.T.astype(np.float32))
    wvt = np.ascontiguousarray(wv.T.astype(np.float32))
    bq = bq.astype(np.float32)
    bk = bk.astype(np.float32)
    bqk = np.ascontiguousarray(
        np.stack([bq[:128], bq[128:], bk[:128], bk[128:]], axis=1))
    mask = np.concatenate([np.zeros((128, 128), np.float32),
                           np.triu(np.ones((128, 128), np.float32))], axis=1)
    common = dict(pe=pe, wqt=wqt, wkt=wkt, wvt=wvt, bqk=bqk,
                  bv=np.ascontiguousarray(bv.astype(np.float32)),
                  mask=mask)
    return [dict(x=np.ascontiguousarray(x_r[i * NB:(i + 1) * NB]), **common)
            for i in range(NCORES)]


_NC_CACHE = None


def kernel(x, wq, bq, wk, bk, wv, bv):
    global _NC_CACHE
    if _NC_CACHE is None:
        _NC_CACHE = build()
    nc = _NC_CACHE
    in_maps = make_in_maps(x, wq, bq, wk, bk, wv, bv)
    res = run_bass_kernel_spmd(nc, in_maps, core_ids=list(range(NCORES)))
    out = np.concatenate([res.results[i]["out"] for i in range(NCORES)], axis=0)
    return np.ascontiguousarray(out.reshape(B, 256, L, L).astype(np.float32))



# revision 2
# speedup vs baseline: 1.0349x; 1.0349x over previous
"""Trainium2 Bass kernel v2: causal spatial attention block, fp8-DoubleRow.

Data-parallel over batch across 8 NeuronCores (4 batches/core, no
collectives). PE plan per batch (cost = out-free-size x cyc/row; fp8
DoubleRow = 0.5 cyc/row and contracts two 128-deep k-tiles per instr):
  - x (with pos-emb channels pre-merged host-side) is DMA'd straight into
    an f32r-typed [128,2,S] tile; the fp8 copy for the Q/K path converts
    from it (DVE+Pool, one half each).
  - Q/K projections: fp8-DR, 16 matmuls of N=256 into [128,2,256] psum
    pairs; relu+bias+fp8-quantize in 8 tensor_scalar ops of 512 cols.
  - V projection: f32r (v-path needs >=bf16 accuracy; fp8 fails the 2e-2
    gate) + fp8-DR rank-1 bias (lhsT 0.25, rhs 4*bv).
  - vT stored SPLIT-fp8: hi = relu(psum) as fp8, lo = (psum max 0) - hi
    (one scalar_tensor_tensor). o-psum accumulates vt_hi@p + vt_lo@p,
    bf16-grade accuracy at fp8-DR speed.
  - scores: fp8-DR per (t-pair, 256-col sub-chunk); causal masking is
    ADDED on the PE (identity-lhsT DR matmuls of -240 tri/full tiles) so
    exp underflows to exact fp8 zeros.
  - p = exp(scores/16): ACT writes fp8 directly, one instr per pair tile.
  - o/dn accumulate region-wise into 512-wide acc psums (o0,o1,dn =
    3 banks; score/proj pairs are 1-bank tiles, bufs=5 -> 8 banks total).
  - denom reciprocal: DVE InstReciprocal (IEEE-exact); osb = o * rec.
  - outputs staged in one [128,2,S] f32 tile, DMA'd once per half per
    batch on the two HWDGE queues (SP + ACT) to dodge the Pool SWDGE tax.
"""
import numpy as np
from contextlib import ExitStack

import concourse.bass as bass
import concourse.mybir as mybir
import concourse.tile as tile
from concourse import bacc
from concourse.bass_utils import run_bass_kernel_spmd

F32 = mybir.dt.float32
F32R = mybir.dt.float32r
FP8 = mybir.dt.float8e4
AF = mybir.ActivationFunctionType
ALU = mybir.AluOpType
DR = mybir.MatmulPerfMode.DoubleRow

B, C, L, EMB = 32, 224, 32, 16
S = L * L            # 1024
CIN = 256
NCORES = 8
NB = B // NCORES     # 4 batches per core
MASKV = -240.0       # fp8e4 max finite; exp((score-240)/16) -> fp8 zero


def _pos_embeddings() -> np.ndarray:
    pos = np.arange(L)[:, None].astype(np.float64)
    j = np.arange(EMB)[None, :]
    enc = pos / np.power(10000.0, 2.0 * (j // 2) / EMB)
    enc[0, :] = 0.0
    enc[1:, 0::2] = np.sin(enc[1:, 0::2])
    enc[1:, 1::2] = np.cos(enc[1:, 1::2])
    t = enc.astype(np.float32)
    x = np.tile(t.reshape(1, EMB, L, 1), (1, 1, 1, L))
    y = np.tile(t.reshape(1, EMB, 1, L), (1, 1, L, 1))
    pe = np.concatenate((x, y), axis=1)[0]
    return np.ascontiguousarray(pe.reshape(2 * EMB, S))


def _pin_act_tables():
    from concourse import bacc as _bacc
    real = _bacc.get_activation_tables
    def patched(arch):
        tables = real(arch)
        keep = "natural_log_exp_and_others"
        assert keep in tables
        return {name: (funcs if name == keep else set())
                for name, funcs in tables.items()}
    _bacc.get_activation_tables = patched
    return real


def build(reps: int = 1):
    real_tables = _pin_act_tables()
    nc = bacc.Bacc("TRN2", target_bir_lowering=False, debug=False,
                   num_devices=NCORES)
    # x has pe channels pre-merged host-side: [NB, 256, S], consumed as f32r
    x_d = nc.declare_dram_parameter("xm", [NB, 2, 128, S], F32R, isOutput=False)
    wq_d = nc.declare_dram_parameter("wqt", [CIN, 256], F32, isOutput=False)
    wk_d = nc.declare_dram_parameter("wkt", [CIN, 256], F32, isOutput=False)
    wv_d = nc.declare_dram_parameter("wvt", [CIN, 256], F32R, isOutput=False)
    bqk_d = nc.declare_dram_parameter("bqk", [128, 4], F32, isOutput=False)
    bv_d = nc.declare_dram_parameter("bv", [256], F32, isOutput=False)
    # mconst: [identity | strict-lower-tri * -240]
    mk_d = nc.declare_dram_parameter("mconst", [128, 256], F32, isOutput=False)
    out_d = nc.declare_dram_parameter("out", [NB, 256, S], F32, isOutput=True)

    with ExitStack() as ctx:
        tc = ctx.enter_context(tile.TileContext(nc))
        const = ctx.enter_context(tc.tile_pool(name="const", bufs=1))
        xrp = ctx.enter_context(tc.tile_pool(name="xr", bufs=2))
        x8p = ctx.enter_context(tc.tile_pool(name="x8", bufs=2))
        qkp = ctx.enter_context(tc.tile_pool(name="qk", bufs=2))
        vtp = ctx.enter_context(tc.tile_pool(name="vt", bufs=2))
        pp = ctx.enter_context(tc.tile_pool(name="p", bufs=6))
        ep = ctx.enter_context(tc.tile_pool(name="epi", bufs=3))
        op = ctx.enter_context(tc.tile_pool(name="ob", bufs=2))
        ps_p = ctx.enter_context(tc.tile_pool(name="pair", bufs=5, space="PSUM"))
        ps_acc = ctx.enter_context(tc.tile_pool(name="acc", bufs=3, space="PSUM"))

        consts = {}

        def _load_wv():
            wvr = const.tile([128, 2, 256], F32R, tag="wv_r")
            nc.sync.dma_start(wvr[:], wv_d[:].rearrange("(ci p) m -> p ci m", p=128))
            consts["wv"] = wvr
            bvf = const.tile([1, 256], F32, tag="bv_f")
            # ACT HWDGE queue: startup DMAs run parallel to the SP queue
            nc.scalar.dma_start(bvf[:], bv_d[:].unsqueeze(0))
            b1 = const.tile([1, 2, 128], FP8, tag="b1")
            nc.vector.memset(b1[:], 0.0)
            nc.vector.memset(b1[0:1, 0, :], 0.25)
            consts["b1"] = b1
            bv8 = const.tile([1, 2, 256], FP8, tag="bv8")
            nc.vector.memset(bv8[:], 0.0)
            nc.vector.tensor_scalar(bv8[0:1, 0, :], bvf[:], 4.0, None,
                                    op0=ALU.mult)
            consts["bv8"] = bv8

        def _make_ones_and_warm():
            ones8 = const.tile([128, 2, 128], FP8, tag="ones8")
            nc.vector.memset(ones8[:], 1.0)
            consts["ones8"] = ones8
            warm = ps_acc.tile([128, 512], F32, tag="acc")
            for _ in range(8):
                nc.tensor.matmul(warm[:, 0:128], ones8[:, 0, :], ones8[:, 0, :],
                                 start=True, stop=True)

        def _load_weights():
            for nm, dram in (("wq", wq_d), ("wk", wk_d)):
                wf = const.tile([128, 2, 256], F32, tag=nm + "_f")
                nc.scalar.dma_start(wf[:], dram[:].rearrange("(ci p) m -> p ci m", p=128))
                w8 = const.tile([128, 2, 256], FP8, tag=nm + "_8")
                nc.vector.tensor_copy(w8[:], wf[:])
                consts[nm] = w8
            bqk = const.tile([128, 4], F32, tag="bqk")
            nc.scalar.dma_start(bqk[:], bqk_d[:])
            consts["bqk"] = bqk
            mcf = const.tile([128, 256], F32, tag="mc_f")
            nc.scalar.dma_start(mcf[:], mk_d[:])
            i8 = const.tile([128, 2, 128], FP8, tag="i8")
            nc.gpsimd.memset(i8[:], 0.0)
            nc.gpsimd.tensor_copy(i8[:, 0, :], mcf[:, 0:128])
            consts["i8"] = i8
            m8 = const.tile([128, 2, 128], FP8, tag="m8")
            nc.gpsimd.memset(m8[:], 0.0)
            nc.gpsimd.tensor_copy(m8[:, 0, :], mcf[:, 128:256])
            consts["m8"] = m8
            fm8 = const.tile([128, 2, 256], FP8, tag="fm8")
            nc.gpsimd.memset(fm8[:], 0.0)
            nc.gpsimd.memset(fm8[:, 0, 0:128], MASKV)
            nc.gpsimd.tensor_copy(fm8[:, 0, 128:256], mcf[:, 128:256])
            consts["fm8"] = fm8

        def load_x0(b):
            if "wv" not in consts:
                _load_wv()
                _make_ones_and_warm()
            x0r = xrp.tile([128, 2, S], F32R, tag="x0r")
            x08 = x8p.tile([128, 2, S], FP8, tag="x08")
            for hi, (h0, h1) in enumerate(((0, 512), (512, S))):
                nc.sync.dma_start(
                    x0r[:, :, h0:h1],
                    x_d[b, :, :, h0:h1].rearrange("sl p s -> p sl s"))
                # fp8 copy on Pool: SBUF->SBUF is the only op class the
                # Pool engine supports in this toolchain (no PSUM reads,
                # no scalar_tensor_tensor), so give it all of this one
                nc.gpsimd.tensor_copy(x08[:, :, h0:h1], x0r[:, :, h0:h1])
            if "wq" not in consts:
                _load_weights()
            return x08, x0r

        x_next = load_x0(0)

        seq = [b for _ in range(reps) for b in range(NB)]
        for bi, b in enumerate(seq):
            is_last = (bi == len(seq) - 1)
            x08, x0r = x_next

            wq8, wk8, wvr = consts["wq"], consts["wk"], consts["wv"]
            bqk, b1, bv8 = consts["bqk"], consts["b1"], consts["bv8"]
            ones8, i8, m8, fm8 = (consts["ones8"], consts["i8"],
                                  consts["m8"], consts["fm8"])

            # ---- V projection -> split-fp8 vT (hi + lo), ti-pairs ----
            vth = vtp.tile([128, 8, 256], FP8, tag="vth")
            vtl = vtp.tile([128, 8, 256], FP8, tag="vtl")
            vt_done = [0]

            def ensure_vt(npairs):
                for a in range(vt_done[0], npairs):
                    vp = ps_p.tile([128, 2, 256], F32, tag="pair")
                    for sl in range(2):
                        ti = 2 * a + sl
                        ts = slice(ti * 128, (ti + 1) * 128)
                        nc.tensor.matmul(vp[:, sl, :], x0r[:, 0, ts],
                                         wvr[:, 0, :], start=True, stop=False)
                        nc.tensor.matmul(vp[:, sl, :], x0r[:, 1, ts],
                                         wvr[:, 1, :], start=False, stop=False)
                        nc.tensor.matmul(vp[:, sl, :], b1[:], bv8[:],
                                         start=False, stop=True, perf_mode=DR)
                    dst_h = vth[:, 2 * a:2 * a + 2, :]
                    dst_l = vtl[:, 2 * a:2 * a + 2, :]
                    # Pool cannot read PSUM on hw: hi on ACT, lo (stt) on DVE
                    nc.scalar.activation(dst_h, vp[:], AF.Relu)
                    nc.vector.scalar_tensor_tensor(dst_l, vp[:], 0.0, dst_h,
                                                   op0=ALU.max,
                                                   op1=ALU.subtract)
                vt_done[0] = max(vt_done[0], npairs)

            # ---- Q/K projections for one s-half (fp8 DoubleRow) ----
            q8t = qkp.tile([128, 2, S], FP8, tag="q8")
            k8t = qkp.tile([128, 2, S], FP8, tag="k8")

            def qk_proj(sj):
                ss = slice(sj * 512, (sj + 1) * 512)
                for pi, (w8, dst) in enumerate(((wq8, q8t), (wk8, k8t))):
                    for m in range(2):
                        qp = ps_p.tile([128, 2, 256], F32, tag="pair")
                        for sq in range(2):
                            s0 = sj * 512 + sq * 256
                            nc.tensor.matmul(
                                qp[:, sq, :],
                                w8[:, :, m * 128:(m + 1) * 128],
                                x08[:, :, s0:s0 + 256],
                                start=True, stop=True, perf_mode=DR)
                        bias_ap = bqk[:, 2 * pi + m:2 * pi + m + 1]
                        dst_ap = dst[:, m, ss].rearrange("p (a q) -> p a q", a=2)
                        if (2 * pi + m) % 2 == 0:
                            nc.scalar.activation(dst_ap, qp[:], AF.Relu,
                                                 bias=bias_ap)
                        else:
                            nc.vector.tensor_scalar(dst_ap, qp[:], bias_ap,
                                                    0.0, op0=ALU.add,
                                                    op1=ALU.max)

            # ---- attention sub-chunk: t-pair a, s columns [sc0, sc0+256) ----
            osb_all = op.tile([128, 2, S], F32, tag="osb")

            # scores+exp for one (t-pair a, 256-col sub-chunk); o/dn matmuls
            # are DEFERRED 2 iterations (via pending queue) so the PE never
            # sits behind an exp it just triggered.
            pending = []

            def attn_scores(a, sub):
                sc0 = sub * 256
                diag = (a == sub)
                sp = ps_p.tile([128, 2, 256], F32, tag="pair")
                for sl in range(2):
                    ti = 2 * a + sl
                    nc.tensor.matmul(sp[:, sl, :],
                                     k8t[:, :, ti * 128:(ti + 1) * 128],
                                     q8t[:, :, sc0:sc0 + 256],
                                     start=True, stop=not diag, perf_mode=DR)
                    if diag:
                        # close this slot's group before slot1 opens its own
                        # (one pending group per psum zero region)
                        mask_rhs = m8 if sl == 0 else fm8
                        mask_out = sp[:, 0, 0:128] if sl == 0 else sp[:, 1, :]
                        nc.tensor.matmul(mask_out, i8[:], mask_rhs[:],
                                         start=False, stop=True, perf_mode=DR)
                pt = pp.tile([128, 2, 256], FP8, tag="pt")
                nc.scalar.activation(pt[:], sp[:], AF.Exp, scale=0.0625)
                return pt

            def epilogue(half):
                ck0 = half * 512
                o0, o1, dn = accs[half]
                rec = ep.tile([128, 512], F32, tag="rec")
                nc.vector.reciprocal(rec[:], dn[:])
                for m, om in enumerate((o0, o1)):
                    nc.vector.tensor_tensor(osb_all[:, m, ck0:ck0 + 512],
                                            om[:], rec[:], op=ALU.mult)

            def attn_odn(job):
                pt, a, sub, o0, o1, dn, first, last, ck0 = job
                reg = slice(sub * 256 - ck0, sub * 256 - ck0 + 256)
                nc.tensor.matmul(dn[:, reg], ones8[:], pt[:],
                                 start=first, stop=last, perf_mode=DR)
                nc.tensor.matmul(o0[:, reg], vth[:, 2 * a:2 * a + 2, 0:128],
                                 pt[:], start=first, stop=False, perf_mode=DR)
                nc.tensor.matmul(o0[:, reg], vtl[:, 2 * a:2 * a + 2, 0:128],
                                 pt[:], start=False, stop=last, perf_mode=DR)
                nc.tensor.matmul(o1[:, reg], vth[:, 2 * a:2 * a + 2, 128:256],
                                 pt[:], start=first, stop=False, perf_mode=DR)
                nc.tensor.matmul(o1[:, reg], vtl[:, 2 * a:2 * a + 2, 128:256],
                                 pt[:], start=False, stop=last, perf_mode=DR)
                if last and sub % 2 == 1:
                    # that was the final o/dn of this 512-chunk: its epilogue
                    # can run on DVE/Pool while the PE continues
                    epilogue(sub // 2)

            def attn_push(job):
                pending.append(job)
                if len(pending) > 3:
                    attn_odn(pending.pop(0))

            def attn_flush():
                while pending:
                    attn_odn(pending.pop(0))

            # ---- batch schedule ----
            # qk first: the scores path (qk matmul -> relu -> scores -> exp)
            # is the longest cross-engine chain; vt is only needed 2
            # deferred iterations later
            qk_proj(0)
            ensure_vt(2)
            accs = {}
            for half in range(2):
                if half == 1:
                    qk_proj(1)
                    ensure_vt(4)
                ck0 = half * 512
                o0 = ps_acc.tile([128, 512], F32, tag="acc")
                o1 = ps_acc.tile([128, 512], F32, tag="acc")
                dn = ps_acc.tile([128, 512], F32, tag="acc")
                accs[half] = (o0, o1, dn)
                for sub in (2 * half, 2 * half + 1):
                    if sub == 3 and not is_last:
                        # prefetch + fp8-convert next batch now: the ACT/DVE
                        # copies finish during this batch's tail, so the
                        # next batch's qk matmuls start without waiting
                        x_next = load_x0(seq[bi + 1])
                    for a in range(sub + 1):
                        pt = attn_scores(a, sub)
                        attn_push((pt, a, sub, o0, o1, dn,
                                   a == 0, a == sub, ck0))
            attn_flush()
            # output DMA: both halves on the SP HWDGE queue (Pool SWDGE
            # costs Pool engine time; ACT is saturated by exp). Last batch:
            # split across SP+ACT queues so the tail drains in parallel.
            if is_last:
                for m in range(2):
                    nc.sync.dma_start(out_d[b, m * 128:(m + 1) * 128, 0:512],
                                      osb_all[:, m, 0:512])
                    nc.scalar.dma_start(out_d[b, m * 128:(m + 1) * 128, 512:],
                                        osb_all[:, m, 512:])
            else:
                nc.sync.dma_start(out_d[b, 0:128, :], osb_all[:, 0, :])
                nc.sync.dma_start(out_d[b, 128:256, :], osb_all[:, 1, :])

    try:
        nc.finalize()
    finally:
        from concourse import bacc as _bacc
        _bacc.get_activation_tables = real_tables
    return nc


def make_in_maps(x, wq, bq, wk, bk, wv, bv):
    x_r = x.reshape(B, C, S).astype(np.float32)
    pe = _pos_embeddings()
    xm = np.concatenate(
        [x_r, np.broadcast_to(pe[None], (B, 2 * EMB, S))], axis=1)
    xm = np.ascontiguousarray(xm.reshape(B, 2, 128, S))
    wqt = np.ascontiguousarray(wq.T.astype(np.float32))
    wkt = np.ascontiguousarray(wk.T.astype(np.float32))
    wvt = np.ascontiguousarray(wv.T.astype(np.float32))
    bq = bq.astype(np.float32)
    bk = bk.astype(np.float32)
    bqk = np.ascontiguousarray(
        np.stack([bq[:128], bq[128:], bk[:128], bk[128:]], axis=1))
    ident = np.eye(128, dtype=np.float32)
    smask = np.tril(np.ones((128, 128), np.float32), k=-1) * MASKV
    mconst = np.ascontiguousarray(np.concatenate([ident, smask], axis=1))
    common = dict(wqt=wqt, wkt=wkt, wvt=wvt, bqk=bqk,
                  bv=np.ascontiguousarray(bv.astype(np.float32)),
                  mconst=mconst)
    return [dict(xm=np.ascontiguousarray(xm[i * NB:(i + 1) * NB]), **common)
            for i in range(NCORES)]


_NC_CACHE = None


def kernel(x, wq, bq, wk, bk, wv, bv):
    global _NC_CACHE
    if _NC_CACHE is None:
        _NC_CACHE = build()
    nc = _NC_CACHE
    in_maps = make_in_maps(x, wq, bq, wk, bk, wv, bv)
    res = run_bass_kernel_spmd(nc, in_maps, core_ids=list(range(NCORES)))
    out = np.concatenate([res.results[i]["out"] for i in range(NCORES)], axis=0)
    return np.ascontiguousarray(out.reshape(B, 256, L, L).astype(np.float32))


# revision 21
# speedup vs baseline: 1.5813x; 1.5280x over previous
"""Trainium2 Bass kernel v2: causal spatial attention block, fp8-DoubleRow.

Data-parallel over batch across 8 NeuronCores (4 batches/core, no
collectives). PE plan per batch (cost = out-free-size x cyc/row; fp8
DoubleRow = 0.5 cyc/row and contracts two 128-deep k-tiles per instr):
  - x (with pos-emb channels pre-merged host-side) is DMA'd straight into
    an f32r-typed [128,2,S] tile; the fp8 copy for the Q/K path converts
    from it (DVE+Pool, one half each).
  - Q/K projections: fp8-DR, 16 matmuls of N=256 into [128,2,256] psum
    pairs; relu+bias+fp8-quantize in 8 tensor_scalar ops of 512 cols.
  - V projection: f32r (v-path needs >=bf16 accuracy; fp8 fails the 2e-2
    gate) + fp8-DR rank-1 bias (lhsT 0.25, rhs 4*bv).
  - vT stored SPLIT-fp8: hi = relu(psum) as fp8, lo = (psum max 0) - hi
    (one scalar_tensor_tensor). o-psum accumulates vt_hi@p + vt_lo@p,
    bf16-grade accuracy at fp8-DR speed.
  - scores: fp8-DR per (t-pair, 256-col sub-chunk); causal masking is
    ADDED on the PE (identity-lhsT DR matmuls of -240 tri/full tiles) so
    exp underflows to exact fp8 zeros.
  - p = exp(scores/16): ACT writes fp8 directly, one instr per pair tile.
  - o/dn accumulate region-wise into 512-wide acc psums (o0,o1,dn =
    3 banks; score/proj pairs are 1-bank tiles, bufs=5 -> 8 banks total).
  - denom reciprocal: DVE InstReciprocal (IEEE-exact); osb = o * rec.
  - outputs staged in one [128,2,S] f32 tile, DMA'd once per half per
    batch on the two HWDGE queues (SP + ACT) to dodge the Pool SWDGE tax.
"""
import numpy as np
from contextlib import ExitStack

import concourse.bass as bass
import concourse.mybir as mybir
import concourse.tile as tile
from concourse import bacc
from concourse.bass_utils import run_bass_kernel_spmd

F32 = mybir.dt.float32
F32R = mybir.dt.float32r
FP8 = mybir.dt.float8e4
AF = mybir.ActivationFunctionType
ALU = mybir.AluOpType
DR = mybir.MatmulPerfMode.DoubleRow

B, C, L, EMB = 32, 224, 32, 16
S = L * L            # 1024
CIN = 256
NCORES = 8
NB = B // NCORES     # 4 batches per core
MASKV = -240.0       # fp8e4 max finite; exp((score-240)/16) -> fp8 zero


def _pos_embeddings() -> np.ndarray:
    pos = np.arange(L)[:, None].astype(np.float64)
    j = np.arange(EMB)[None, :]
    enc = pos / np.power(10000.0, 2.0 * (j // 2) / EMB)
    enc[0, :] = 0.0
    enc[1:, 0::2] = np.sin(enc[1:, 0::2])
    enc[1:, 1::2] = np.cos(enc[1:, 1::2])
    t = enc.astype(np.float32)
    x = np.tile(t.reshape(1, EMB, L, 1), (1, 1, 1, L))
    y = np.tile(t.reshape(1, EMB, 1, L), (1, 1, L, 1))
    pe = np.concatenate((x, y), axis=1)[0]
    return np.ascontiguousarray(pe.reshape(2 * EMB, S))


def _pin_act_tables():
    from concourse import bacc as _bacc
    real = _bacc.get_activation_tables
    def patched(arch):
        tables = real(arch)
        keep = "natural_log_exp_and_others"
        assert keep in tables
        return {name: (funcs if name == keep else set())
                for name, funcs in tables.items()}
    _bacc.get_activation_tables = patched
    return real


def build(reps: int = 1):
    real_tables = _pin_act_tables()
    nc = bacc.Bacc("TRN2", target_bir_lowering=False, debug=False,
                   num_devices=NCORES)
    # x has pe channels pre-merged host-side: [NB, 256, S], consumed as f32r
    x_d = nc.declare_dram_parameter("xm", [NB, 2, 128, S], F32R, isOutput=False)
    wq_d = nc.declare_dram_parameter("wqt", [CIN, 256], F32, isOutput=False)
    wk_d = nc.declare_dram_parameter("wkt", [CIN, 256], F32, isOutput=False)
    wv_d = nc.declare_dram_parameter("wvt", [CIN, 256], F32R, isOutput=False)
    bqk_d = nc.declare_dram_parameter("bqk", [128, 4], F32, isOutput=False)
    bv_d = nc.declare_dram_parameter("bv", [256], F32, isOutput=False)
    # mconst: [identity | strict-lower-tri * -240]
    mk_d = nc.declare_dram_parameter("mconst", [128, 256], F32, isOutput=False)
    out_d = nc.declare_dram_parameter("out", [NB, 256, S], F32, isOutput=True)

    with ExitStack() as ctx:
        tc = ctx.enter_context(tile.TileContext(nc))
        const = ctx.enter_context(tc.tile_pool(name="const", bufs=1))
        xrp = ctx.enter_context(tc.tile_pool(name="xr", bufs=2))
        x8p = ctx.enter_context(tc.tile_pool(name="x8", bufs=2))
        qkp = ctx.enter_context(tc.tile_pool(name="qk", bufs=2))
        vtp = ctx.enter_context(tc.tile_pool(name="vt", bufs=2))
        pp = ctx.enter_context(tc.tile_pool(name="p", bufs=12))
        ep = ctx.enter_context(tc.tile_pool(name="epi", bufs=3))
        op = ctx.enter_context(tc.tile_pool(name="ob", bufs=2))
        ps_p = ctx.enter_context(tc.tile_pool(name="pair", bufs=5, space="PSUM"))
        ps_acc = ctx.enter_context(tc.tile_pool(name="acc", bufs=3, space="PSUM"))

        consts = {}

        def _load_wv():
            wvr = const.tile([128, 2, 256], F32R, tag="wv_r")
            nc.sync.dma_start(wvr[:], wv_d[:].rearrange("(ci p) m -> p ci m", p=128))
            consts["wv"] = wvr
            b1 = const.tile([1, 2, 128], FP8, tag="b1")
            nc.vector.memset(b1[:], 0.0)
            nc.vector.memset(b1[0:1, 0, :], 0.25)
            consts["b1"] = b1
            bv8 = const.tile([1, 2, 256], FP8, tag="bv8")
            nc.vector.memset(bv8[:], 0.0)
            # SWDGE casting DMA: f32 DRAM -> fp8 SBUF, bit-exact RNE
            # (host ships bv pre-scaled by 4)
            nc.gpsimd.dma_start(bv8[0:1, 0, :], bv_d[:].unsqueeze(0))
            consts["bv8"] = bv8


        def _make_ones_and_warm():
            ones8 = const.tile([128, 2, 128], FP8, tag="ones8")
            nc.vector.memset(ones8[:], 1.0)
            consts["ones8"] = ones8
            warm = ps_acc.tile([128, 512], F32, tag="acc")
            for _ in range(8):
                nc.tensor.matmul(warm[:, 0:128], ones8[:, 0, :], ones8[:, 0, :],
                                 start=True, stop=True)

        def _load_weights():
            # SWDGE casting DMAs (f32 DRAM -> fp8 SBUF, RNE): no staging
            # tiles, no DVE/Pool conversion passes, Pool is idle at startup
            for nm, dram in (("wq", wq_d), ("wk", wk_d)):
                w8 = const.tile([128, 2, 256], FP8, tag=nm + "_8")
                nc.gpsimd.dma_start(
                    w8[:], dram[:].rearrange("(ci p) m -> p ci m", p=128))
                consts[nm] = w8
            bqk = const.tile([128, 4], F32, tag="bqk")
            nc.scalar.dma_start(bqk[:], bqk_d[:])
            consts["bqk"] = bqk
            i8 = const.tile([128, 2, 128], FP8, tag="i8")
            nc.gpsimd.memset(i8[:], 0.0)
            nc.gpsimd.dma_start(i8[:, 0, :], mk_d[:, 0:128])
            consts["i8"] = i8
            m8 = const.tile([128, 2, 128], FP8, tag="m8")
            nc.gpsimd.memset(m8[:], 0.0)
            nc.gpsimd.dma_start(m8[:, 0, :], mk_d[:, 128:256])
            consts["m8"] = m8
            fm8 = const.tile([128, 2, 256], FP8, tag="fm8")
            nc.gpsimd.memset(fm8[:], 0.0)
            nc.gpsimd.memset(fm8[:, 0, 0:128], MASKV)
            nc.gpsimd.dma_start(fm8[:, 0, 128:256], mk_d[:, 128:256])
            consts["fm8"] = fm8

        def load_x0(b):
            first = "wv" not in consts
            x0r = xrp.tile([128, 2, S], F32R, tag="x0r")
            x08 = x8p.tile([128, 2, S], FP8, tag="x08")
            for hi, (h0, h1) in enumerate(((0, 512), (512, S))):
                nc.sync.dma_start(
                    x0r[:, :, h0:h1],
                    x_d[b, :, :, h0:h1].rearrange("sl p s -> p sl s"))
                if first and hi == 0:
                    # batch 0: x half-0 heads the SP queue (the qk/scores
                    # chain gates startup); wv can land after it since the
                    # V path is consumed via the deferred o/dn queue
                    _load_wv()
                    _make_ones_and_warm()
                # fp8 copy on Pool: SBUF->SBUF is the only op class the
                # Pool engine supports in this toolchain (no PSUM reads,
                # no scalar_tensor_tensor), so give it all of this one
                nc.gpsimd.tensor_copy(x08[:, :, h0:h1], x0r[:, :, h0:h1])
            if "wq" not in consts:
                _load_weights()
            return x08, x0r

        x_next = load_x0(0)

        # o/dn jobs are deferred in a queue that DRAINS ACROSS BATCH
        # BOUNDARIES: batch b's tail o/dn + epilogue interleave with batch
        # b+1's projections instead of bunching at the boundary. Jobs carry
        # all batch-local tiles explicitly (no closure capture).
        pending = []

        def attn_odn(job):
            (pt, a, sub, o0, o1, dn, first, last, ck0, vth, vtl,
             osb_all, b, last_batch) = job
            reg = slice(sub * 256 - ck0, sub * 256 - ck0 + 256)
            nc.tensor.matmul(dn[:, reg], consts["ones8"][:], pt[:],
                             start=first, stop=last, perf_mode=DR)
            nc.tensor.matmul(o0[:, reg], vth[:, 2 * a:2 * a + 2, 0:128],
                             pt[:], start=first, stop=False, perf_mode=DR)
            nc.tensor.matmul(o0[:, reg], vtl[:, 2 * a:2 * a + 2, 0:128],
                             pt[:], start=False, stop=last, perf_mode=DR)
            nc.tensor.matmul(o1[:, reg], vth[:, 2 * a:2 * a + 2, 128:256],
                             pt[:], start=first, stop=False, perf_mode=DR)
            nc.tensor.matmul(o1[:, reg], vtl[:, 2 * a:2 * a + 2, 128:256],
                             pt[:], start=False, stop=last, perf_mode=DR)
            if not (last and sub % 2 == 1):
                return
            # final o/dn of a 512-chunk: normalize on DVE while PE continues
            rec = ep.tile([128, 512], F32, tag="rec")
            nc.vector.reciprocal(rec[:], dn[:])
            for m, om in enumerate((o0, o1)):
                nc.vector.tensor_tensor(osb_all[:, m, ck0:ck0 + 512],
                                        om[:], rec[:], op=ALU.mult)
            if sub == 3:
                # whole batch normalized: ship it
                if last_batch:
                    for m in range(2):
                        nc.sync.dma_start(
                            out_d[b, m * 128:(m + 1) * 128, 0:512],
                            osb_all[:, m, 0:512])
                        nc.scalar.dma_start(
                            out_d[b, m * 128:(m + 1) * 128, 512:],
                            osb_all[:, m, 512:])
                else:
                    nc.sync.dma_start(out_d[b, 0:128, :], osb_all[:, 0, :])
                    nc.sync.dma_start(out_d[b, 128:256, :], osb_all[:, 1, :])

        def attn_push(job):
            pending.append(job)
            limit = 3 if job[-1] else 10
            if len(pending) > limit:
                attn_odn(pending.pop(0))

        def attn_flush():
            while pending:
                attn_odn(pending.pop(0))

        seq = [b for _ in range(reps) for b in range(NB)]
        for bi, b in enumerate(seq):
            is_last = (bi == len(seq) - 1)
            x08, x0r = x_next

            wq8, wk8, wvr = consts["wq"], consts["wk"], consts["wv"]
            bqk, b1, bv8 = consts["bqk"], consts["b1"], consts["bv8"]
            ones8, i8, m8, fm8 = (consts["ones8"], consts["i8"],
                                  consts["m8"], consts["fm8"])

            # ---- V projection -> split-fp8 vT (hi + lo), ti-pairs ----
            vth = vtp.tile([128, 8, 256], FP8, tag="vth")
            vtl = vtp.tile([128, 8, 256], FP8, tag="vtl")
            vt_done = [0]

            def ensure_vt(npairs):
                for a in range(vt_done[0], npairs):
                    vp = ps_p.tile([128, 2, 256], F32, tag="pair")
                    for sl in range(2):
                        ti = 2 * a + sl
                        ts = slice(ti * 128, (ti + 1) * 128)
                        nc.tensor.matmul(vp[:, sl, :], x0r[:, 0, ts],
                                         wvr[:, 0, :], start=True, stop=False)
                        nc.tensor.matmul(vp[:, sl, :], x0r[:, 1, ts],
                                         wvr[:, 1, :], start=False, stop=False)
                        nc.tensor.matmul(vp[:, sl, :], b1[:], bv8[:],
                                         start=False, stop=True, perf_mode=DR)
                    # drain relu(v) ONCE to bf16 SBUF (ACT/DVE are the
                    # psum-drain bottleneck pair); the fp8 hi/lo split is
                    # then pure SBUF work on the otherwise-idle Pool
                    dst_r = rv[:, 2 * a:2 * a + 2, :]
                    dst_h = vth[:, 2 * a:2 * a + 2, :]
                    dst_l = vtl[:, 2 * a:2 * a + 2, :]
                    if a < 3:
                        nc.scalar.activation(dst_r, vp[:], AF.Relu)
                    else:
                        nc.vector.tensor_scalar(dst_r, vp[:], 0.0, None,
                                                op0=ALU.max)
                    nc.gpsimd.tensor_copy(dst_h, dst_r)
                    nc.gpsimd.tensor_tensor(dst_l, dst_r, dst_h,
                                            op=ALU.subtract)
                vt_done[0] = max(vt_done[0], npairs)

            # ---- Q/K projections for one s-half (fp8 DoubleRow) ----
            q8t = qkp.tile([128, 2, S], FP8, tag="q8")
            k8t = qkp.tile([128, 2, S], FP8, tag="k8")

            def qk_proj(sj):
                ss = slice(sj * 512, (sj + 1) * 512)
                for pi, (w8, dst) in enumerate(((wq8, q8t), (wk8, k8t))):
                    for m in range(2):
                        qp = ps_p.tile([128, 2, 256], F32, tag="pair")
                        for sq in range(2):
                            s0 = sj * 512 + sq * 256
                            nc.tensor.matmul(
                                qp[:, sq, :],
                                w8[:, :, m * 128:(m + 1) * 128],
                                x08[:, :, s0:s0 + 256],
                                start=True, stop=True, perf_mode=DR)
                        bias_ap = bqk[:, 2 * pi + m:2 * pi + m + 1]
                        dst_ap = dst[:, m, ss].rearrange("p (a q) -> p a q", a=2)
                        if 2 * pi + m == 0 and sj == 0:
                            nc.scalar.activation(dst_ap, qp[:], AF.Relu,
                                                 bias=bias_ap)
                        else:
                            nc.vector.tensor_scalar(dst_ap, qp[:], bias_ap,
                                                    0.0, op0=ALU.add,
                                                    op1=ALU.max)

            # ---- attention sub-chunk: t-pair a, s columns [sc0, sc0+256) ----
            osb_all = op.tile([128, 2, S], F32, tag="osb")

            def attn_scores(a, sub):
                sc0 = sub * 256
                diag = (a == sub)
                sp = ps_p.tile([128, 2, 256], F32, tag="pair")
                for sl in range(2):
                    ti = 2 * a + sl
                    nc.tensor.matmul(sp[:, sl, :],
                                     k8t[:, :, ti * 128:(ti + 1) * 128],
                                     q8t[:, :, sc0:sc0 + 256],
                                     start=True, stop=not diag, perf_mode=DR)
                    if diag:
                        # close this slot's group before slot1 opens its own
                        # (one pending group per psum zero region)
                        mask_rhs = m8 if sl == 0 else fm8
                        mask_out = sp[:, 0, 0:128] if sl == 0 else sp[:, 1, :]
                        nc.tensor.matmul(mask_out, i8[:], mask_rhs[:],
                                         start=False, stop=True, perf_mode=DR)
                pt = pp.tile([128, 2, 256], FP8, tag="pt")
                nc.scalar.activation(pt[:], sp[:], AF.Exp, scale=0.0625)
                return pt

            # ---- batch schedule ----
            # qk first: the scores path (qk matmul -> relu -> scores -> exp)
            # is the longest cross-engine chain; vt is only needed 2
            # deferred iterations later
            qk_proj(0)
            ensure_vt(2)
            for half in range(2):
                if half == 1:
                    qk_proj(1)
                    ensure_vt(4)
                ck0 = half * 512
                o0 = ps_acc.tile([128, 512], F32, tag="acc")
                o1 = ps_acc.tile([128, 512], F32, tag="acc")
                dn = ps_acc.tile([128, 512], F32, tag="acc")
                for sub in (2 * half, 2 * half + 1):
                    if sub == 3 and not is_last:
                        # prefetch + fp8-convert next batch now: the copies
                        # finish during this batch's tail, so the next
                        # batch's qk matmuls start without waiting
                        x_next = load_x0(seq[bi + 1])
                    for a in range(sub + 1):
                        pt = attn_scores(a, sub)
                        attn_push((pt, a, sub, o0, o1, dn,
                                   a == 0, a == sub, ck0, vth, vtl,
                                   osb_all, b, is_last))
            if is_last:
                attn_flush()

    try:
        nc.finalize()
    finally:
        from concourse import bacc as _bacc
        _bacc.get_activation_tables = real_tables
    return nc


def make_in_maps(x, wq, bq, wk, bk, wv, bv):
    x_r = x.reshape(B, C, S).astype(np.float32)
    pe = _pos_embeddings()
    xm = np.concatenate(
        [x_r, np.broadcast_to(pe[None], (B, 2 * EMB, S))], axis=1)
    xm = np.ascontiguousarray(xm.reshape(B, 2, 128, S))
    wqt = np.ascontiguousarray(wq.T.astype(np.float32))
    wkt = np.ascontiguousarray(wk.T.astype(np.float32))
    wvt = np.ascontiguousarray(wv.T.astype(np.float32))
    bq = bq.astype(np.float32)
    bk = bk.astype(np.float32)
    bqk = np.ascontiguousarray(
        np.stack([bq[:128], bq[128:], bk[:128], bk[128:]], axis=1))
    ident = np.eye(128, dtype=np.float32)
    smask = np.tril(np.ones((128, 128), np.float32), k=-1) * MASKV
    mconst = np.ascontiguousarray(np.concatenate([ident, smask], axis=1))
    common = dict(wqt=wqt, wkt=wkt, wvt=wvt, bqk=bqk,
                  bv=np.ascontiguousarray(4.0 * bv.astype(np.float32)),
                  mconst=mconst)
    return [dict(xm=np.ascontiguousarray(xm[i * NB:(i + 1) * NB]), **common)
            for i in range(NCORES)]


_NC_CACHE = None


def kernel(x, wq, bq, wk, bk, wv, bv):
    global _NC_CACHE
    if _NC_CACHE is None:
        _NC_CACHE = build()
    nc = _NC_CACHE
    in_maps = make_in_maps(x, wq, bq, wk, bk, wv, bv)
    res = run_bass_kernel_spmd(nc, in_maps, core_ids=list(range(NCORES)))
    out = np.concatenate([res.results[i]["out"] for i in range(NCORES)], axis=0)
    return np.ascontiguousarray(out.reshape(B, 256, L, L).astype(np.float32))
